# revision 3
# baseline (speedup 1.0000x reference)
"""Dense correspondence contrastive loss kernel for Trainium2 (8 NeuronCores).

Problem (B=32, C=64, N=1024 spatial positions per sample):
  - l2-normalize q_b/k_b/q_grid/k_grid along C
  - sim[b,i,j] = <qb_hat[b,:,i], kb_hat[b,:,j]>; idx = argmax_j sim
  - pos[b,i] = <qg_hat[b,:,i], kg_hat[b,:,idx[b,i]]> / 0.1
  - neg[b,i] = <qg_hat[b,:,i], kg_hat[neg_idx[b],:,i]> / 0.1
    (neg_idx from labels/neg_noise -- O(B^2) host-side index prep)
  - loss = mean(log(exp(pos)+exp(neg)+1e-6) - pos)

Sharding: data-parallel over batch, 4 samples per core.  Per core the
device does: k_b column norms (PE colsum into a partition-parallel
[8,128] layout + K=1 broadcast matmuls), bf16 sim matmuls into fp32
PSUM, argmax via DVE max-reduce + fused (sim>=max)*iota row-sum
(exact: no fp32 ties), indirect-DMA gather of the matched k_grid rows
(host pre-transposes q_grid/k_grid to [N, C] so rows are contiguous
and channel norms reduce along the free dim), then a batched loss
tail.  Host sums 8 partial scalars.

Key algebraic shortcut: q_b normalization is skipped entirely --
argmax_j over j is invariant to the per-row positive scale 1/|q_b[:,i]|.
bf16 is used only for the sim matmul operands (PSUM accumulates fp32);
measured end-to-end impact ~4e-4 relative, from ~140/32768 argmax
flips between near-equal similarities.
"""

import os
import numpy as np

B = 32
C = 64
N = 1024
NCORES = 8
SPC = B // NCORES          # samples per core
MT = N // 128              # 128-row m-tiles per sample
NT = SPC * MT              # accumulator columns per core
TEMP = 0.1
EPS_LOSS = 1e-6

LAST_EXEC_TIME_NS = None
_CACHE = {}


def _ensure_ntff_hook():
    """Some agent images ship only the antenv stub (no axon_hooks); bass_utils
    then crashes on `from antenv.axon_hooks import ...` when tracing under
    axon.  Install a functional shim wired to the libaxon ctypes hook so NTFF
    profiling (and exec_time_ns) works.  No-op when the real module exists."""
    import sys
    import types
    try:
        import antenv.axon_hooks  # noqa: F401
        return
    except ImportError:
        pass
    try:
        import antenv
    except ImportError:
        return
    mod = types.ModuleType("antenv.axon_hooks")
    mod._hook = None

    def set_axon_ntff_profile_hook(h):
        mod._hook = h

    def get_axon_ntff_profile_hook():
        return mod._hook

    mod.set_axon_ntff_profile_hook = set_axon_ntff_profile_hook
    mod.get_axon_ntff_profile_hook = get_axon_ntff_profile_hook
    sys.modules["antenv.axon_hooks"] = mod
    antenv.axon_hooks = mod
    try:
        from trn_agent_boot.trn_boot import _ntff_profile_via_ctypes
        hook = _ntff_profile_via_ctypes("/opt/axon/libaxon_pjrt.so")
        if hook is not None:
            mod._hook = hook
    except Exception:
        pass


def _build_module():
    import concourse.bass as bass
    import concourse.bacc as bacc
    import concourse.tile as tile
    from concourse import mybir
    from contextlib import ExitStack

    F32 = mybir.dt.float32
    BF16 = mybir.dt.bfloat16
    U32 = mybir.dt.uint32
    AX = mybir.AxisListType
    ALU = mybir.AluOpType
    ACTF = mybir.ActivationFunctionType

    nc = bacc.Bacc("TRN2", target_bir_lowering=False, debug=False,
                   num_devices=NCORES)

    qb_d = nc.dram_tensor("qb", [SPC * C, N], F32, kind="ExternalInput")
    kb_d = nc.dram_tensor("kb", [SPC * C, N], F32, kind="ExternalInput")
    qgt_d = nc.dram_tensor("qgt", [SPC * N, C], F32, kind="ExternalInput")
    kgt_d = nc.dram_tensor("kgt", [SPC * N, C], F32, kind="ExternalInput")
    kngt_d = nc.dram_tensor("kngt", [SPC * N, C], F32, kind="ExternalInput")
    ind_d = nc.dram_tensor("cst_ind", [C, MT * MT], F32, kind="ExternalInput")
    indT_d = nc.dram_tensor("cst_indT", [MT, MT * C], F32, kind="ExternalInput")
    out_d = nc.dram_tensor("out", [1, 1], F32, kind="ExternalOutput")

    with tile.TileContext(nc) as tc, ExitStack() as ctx:
        const = ctx.enter_context(tc.tile_pool(name="const", bufs=1))
        accum = ctx.enter_context(tc.tile_pool(name="accum", bufs=1))
        io = ctx.enter_context(tc.tile_pool(name="io", bufs=3))
        mt_p = ctx.enter_context(tc.tile_pool(name="mt", bufs=4))
        qg_p = ctx.enter_context(tc.tile_pool(name="qg", bufs=2))
        scr = ctx.enter_context(tc.tile_pool(name="scr", bufs=6))
        ps_sim = ctx.enter_context(tc.tile_pool(name="ps_sim", bufs=3, space="PSUM"))
        ps_aux = ctx.enter_context(tc.tile_pool(name="ps_aux", bufs=2, space="PSUM"))

        iota = const.tile([128, N], F32)
        nc.gpsimd.iota(iota[:], pattern=[[1, N]], base=0, channel_multiplier=0,
                       allow_small_or_imprecise_dtypes=True)
        ones64 = const.tile([C, 1], F32)
        nc.vector.memset(ones64[:], 1.0)
        ones1x64 = const.tile([1, C], F32)
        nc.vector.memset(ones1x64[:], 1.0)
        ones128 = const.tile([128, 1], F32)
        nc.vector.memset(ones128[:], 1.0)
        b24 = const.tile([128, 1], F32)
        nc.vector.memset(b24[:], 1e-24)
        # indicator weights: column j ones -> colsum of chunk j accumulates
        # into PSUM partition j (PE output base partition must be 0)
        # indicator weight matrices (host-supplied constants):
        # inds[j] [C,MT]: column j ones -> colsum of chunk j lands in PSUM
        # partition j; indTs[j] [MT,C]: row j ones -> broadcasts chunk j of
        # the [MT,128] reciprocal tile over all C output partitions
        ind_sb = const.tile([C, MT * MT], F32)
        nc.sync.dma_start(ind_sb[:], ind_d[:, :])
        indT_sb = const.tile([MT, MT * C], F32)
        nc.sync.dma_start(indT_sb[:], indT_d[:, :])
        inds = [ind_sb[:, j * MT:(j + 1) * MT] for j in range(MT)]
        indTs = [indT_sb[:, j * C:(j + 1) * C] for j in range(MT)]

        # merged norm accumulators: cols [0:NT)=qg, [NT:2NT)=k_gathered,
        # [2NT:3NT)=k_neg; one sqrt+reciprocal in the tail covers all three
        ssqa = accum.tile([128, 3 * NT], F32, tag="ssqa")
        dps = accum.tile([128, NT], F32, tag="dps")
        dns = accum.tile([128, NT], F32, tag="dns")

        import concourse.bass as bass_mod

        def emit_norm(b):
            """Loads + k_b column-norm chain + pos/neg prep for sample b."""
            st = {}
            kb_t = io.tile([C, N], F32, tag="kb")
            nc.sync.dma_start(kb_t[:], kb_d[b * C:(b + 1) * C, :])
            qb_t = io.tile([C, N], F32, tag="qb")
            nc.sync.dma_start(qb_t[:], qb_d[b * C:(b + 1) * C, :])

            # k_b column norms: colsums land partition-parallel ([8,128], one
            # 128-col chunk per partition) so the reciprocal runs at 128 free
            # elems instead of 1024
            sq = io.tile([C, N], F32, tag="sq")
            nc.scalar.activation(sq[:], kb_t[:], ACTF.Square)
            ssq_ps = ps_aux.tile([MT, 128], F32, tag="aux")
            for j in range(MT):
                nc.tensor.matmul(ssq_ps[:], inds[j],
                                 sq[:, j * 128:(j + 1) * 128],
                                 start=(j == 0), stop=(j == MT - 1))
            rn_s = io.tile([MT, 128], F32, tag="rn_s")
            nc.scalar.activation(rn_s[:], ssq_ps[:], ACTF.Sqrt, bias=b24[0:MT, :])
            rn8 = io.tile([MT, 128], F32, tag="rn8")
            nc.vector.reciprocal(rn8[:], rn_s[:])

            # broadcast 1/|k_j| over C (K=MT matmul per 128-col chunk) and
            # scale k_b, emitting bf16 for the sim matmul
            kbh = io.tile([C, N], BF16, tag="kbh")
            for h in range(2):
                rnb_ps = ps_aux.tile([C, 512], F32, tag="aux")
                for j in range(4):
                    nc.tensor.matmul(rnb_ps[:, j * 128:(j + 1) * 128],
                                     indTs[4 * h + j], rn8[:],
                                     start=True, stop=True)
                nc.vector.tensor_mul(kbh[:, h * 512:(h + 1) * 512],
                                     kb_t[:, h * 512:(h + 1) * 512], rnb_ps[:])
            qb_bf = io.tile([C, N], BF16, tag="qb_bf")
            nc.scalar.activation(qb_bf[:], qb_t[:], ACTF.Copy)
            st["kbh"], st["qb_bf"] = kbh, qb_bf

            # whole-sample strided loads: [128, MT*C] with m-tile m in columns
            # [m*C, (m+1)*C); issued on the ACT HWDGE ring to unload Sync-seq
            qgs = qg_p.tile([128, MT * C], F32, tag="qg")
            nc.scalar.dma_start(
                qgs[:], qgt_d[b * N:(b + 1) * N, :].rearrange("(m p) c -> p m c", p=128))
            kngs = qg_p.tile([128, MT * C], F32, tag="kng")
            nc.scalar.dma_start(
                kngs[:], kngt_d[b * N:(b + 1) * N, :].rearrange("(m p) c -> p m c", p=128))
            st["qgs"], st["kngs"] = qgs, kngs

            # channel sum-squares per m-chunk on ACT (keeps DVE free); raw
            # negative dot products on gpsimd
            for m in range(MT):
                t = b * MT + m
                s1 = scr.tile([128, C], F32, tag="s64")
                nc.scalar.activation(s1[:], qgs[:, m * C:(m + 1) * C], ACTF.Square,
                                     accum_out=ssqa[:, t:t + 1])
                s2 = scr.tile([128, C], F32, tag="s64")
                nc.scalar.activation(s2[:], kngs[:, m * C:(m + 1) * C], ACTF.Square,
                                     accum_out=ssqa[:, 2 * NT + t:2 * NT + t + 1])
            idxs = mt_p.tile([128, MT], F32, tag="idxs")
            st["idxs"] = idxs
            return st

        def emit_mtile(b, m, st):
            sim_ps = ps_sim.tile([128, N], F32, tag="sim")
            nc.tensor.matmul(sim_ps[:, 0:512], st["qb_bf"][:, m * 128:(m + 1) * 128],
                             st["kbh"][:, 0:512], start=True, stop=True)
            nc.tensor.matmul(sim_ps[:, 512:N], st["qb_bf"][:, m * 128:(m + 1) * 128],
                             st["kbh"][:, 512:N], start=True, stop=True)
            gmax = mt_p.tile([128, 1], F32, tag="gmax")
            nc.vector.reduce_max(gmax[:], sim_ps[:], axis=AX.X)
            big = scr.tile([128, N], F32, tag="big")
            nc.vector.scalar_tensor_tensor(
                big[:], sim_ps[:], gmax[:], iota[:],
                op0=ALU.is_ge, op1=ALU.mult, accum_out=st["idxs"][:, m:m + 1])

        def emit_gather(b, st, mlo, mhi):
            # argmax columns -> clamped u32 row indices into the flat
            # [SPC*N, C] transposed k_grid (tie-sum clamp is belt-and-braces;
            # fp32 sims tie with probability ~0)
            idxc = mt_p.tile([128, mhi - mlo], F32, tag="idxc")
            nc.vector.tensor_scalar(idxc[:], st["idxs"][:, mlo:mhi], 1023.0,
                                    float(b * N), op0=ALU.min, op1=ALU.add)
            idxu = mt_p.tile([128, mhi - mlo], U32, tag="idxu")
            nc.vector.tensor_copy(idxu[:], idxc[:])

            kgas = st["kgas"]
            for m in range(mlo, mhi):
                nc.gpsimd.indirect_dma_start(
                    kgas[:, m * C:(m + 1) * C], None, kgt_d.ap(),
                    bass_mod.IndirectOffsetOnAxis(ap=idxu[:, m - mlo:m - mlo + 1], axis=0))
            for m in range(mlo, mhi):
                t = b * MT + m
                s3 = scr.tile([128, C], F32, tag="s64")
                nc.scalar.activation(s3[:], kgas[:, m * C:(m + 1) * C], ACTF.Square,
                                     accum_out=ssqa[:, NT + t:NT + t + 1])
            prodp = st["prodp"]
            nc.gpsimd.tensor_mul(prodp[:, mlo * C:mhi * C],
                                 st["qgs"][:, mlo * C:mhi * C],
                                 kgas[:, mlo * C:mhi * C])
            if mhi == MT:
                # negative-path product rides last so it never gates gathers
                prodn = scr.tile([128, MT * C], F32, tag="prodn")
                nc.gpsimd.tensor_mul(prodn[:], st["qgs"][:], st["kngs"][:])
                st["prodn"] = prodn

        def emit_reduces(b, st):
            # deferred into the next sample's m-tile stream so DVE's in-order
            # execution doesn't stall on the gpsimd gather->product chain
            nc.vector.tensor_reduce(dps[:, b * MT:(b + 1) * MT],
                                    st["prodp"][:].rearrange("p (m c) -> p m c", c=C),
                                    axis=AX.X, op=ALU.add)
            nc.vector.tensor_reduce(dns[:, b * MT:(b + 1) * MT],
                                    st["prodn"][:].rearrange("p (m c) -> p m c", c=C),
                                    axis=AX.X, op=ALU.add)

        # software-pipelined emission: the next sample's norm chain is emitted
        # two m-tiles into the current sample, so each engine's program order
        # interleaves it into otherwise-idle slots instead of serializing it
        # at the sample boundary; dot reduces defer one sample further
        st = emit_norm(0)
        states = {0: st}
        pending = None
        for b in range(SPC):
            cur = states.pop(b)
            cur["kgas"] = qg_p.tile([128, MT * C], F32, tag="kga", name=f"kgas{b}")
            cur["prodp"] = scr.tile([128, MT * C], F32, tag="prodp", name=f"prodp{b}")
            last = b == SPC - 1
            for m in range(MT):
                emit_mtile(b, m, cur)
                if m == 1 and not last:
                    states[b + 1] = emit_norm(b + 1)
                if m == 6 and pending is not None:
                    emit_reduces(b - 1, pending)
                    pending = None
                if last and m in (1, 3, 5):
                    # overlap the final sample's gathers with its own m-tiles
                    emit_gather(b, cur, m - 1, m + 1)
            if last:
                emit_gather(b, cur, 6, MT)
                emit_reduces(b, cur)
            else:
                emit_gather(b, cur, 0, MT)
                pending = cur

        # batched loss tail over the [128, NT] accumulators; the 1/TEMP=10
        # factor rides along as the stt immediate
        ra_s = accum.tile([128, 3 * NT], F32, tag="ra_s")
        nc.scalar.activation(ra_s[:], ssqa[:], ACTF.Sqrt, bias=b24[:])
        ra = accum.tile([128, 3 * NT], F32, tag="ra")
        nc.vector.reciprocal(ra[:], ra_s[:])

        t1 = accum.tile([128, NT], F32, tag="t1")
        nc.vector.tensor_mul(t1[:], dps[:], ra[:, 0:NT])
        pos = accum.tile([128, NT], F32, tag="pos")
        nc.vector.scalar_tensor_tensor(pos[:], t1[:], 10.0, ra[:, NT:2 * NT],
                                       op0=ALU.mult, op1=ALU.mult)
        t2 = accum.tile([128, NT], F32, tag="t2")
        nc.vector.tensor_mul(t2[:], dns[:], ra[:, 0:NT])
        ngv = accum.tile([128, NT], F32, tag="ngv")
        nc.vector.scalar_tensor_tensor(ngv[:], t2[:], 10.0, ra[:, 2 * NT:3 * NT],
                                       op0=ALU.mult, op1=ALU.mult)

        ep = accum.tile([128, NT], F32, tag="ep")
        nc.scalar.activation(ep[:], pos[:], ACTF.Exp)
        en = accum.tile([128, NT], F32, tag="en")
        nc.scalar.activation(en[:], ngv[:], ACTF.Exp)
        ssum = accum.tile([128, NT], F32, tag="ssum")
        nc.vector.scalar_tensor_tensor(ssum[:], ep[:], EPS_LOSS, en[:],
                                       op0=ALU.add, op1=ALU.add)
        lg = accum.tile([128, NT], F32, tag="lg")
        nc.scalar.activation(lg[:], ssum[:], ACTF.Ln)
        li = accum.tile([128, NT], F32, tag="li")
        nc.vector.tensor_sub(li[:], lg[:], pos[:])
        lsum = accum.tile([128, 1], F32, tag="lsum")
        nc.vector.reduce_sum(lsum[:], li[:], axis=AX.X)

        tot_ps = ps_aux.tile([1, 1], F32, tag="aux")
        nc.tensor.matmul(tot_ps[:], lsum[:], ones128[:], start=True, stop=True)
        outt = mt_p.tile([1, 1], F32, tag="outt")
        nc.scalar.activation(outt[:], tot_ps[:], ACTF.Copy)
        nc.sync.dma_start(out_d[:, :], outt[:])

    nc.compile()
    return nc


def get_module():
    if "nc" not in _CACHE:
        _CACHE["nc"] = _build_module()
    return _CACHE["nc"]


def make_in_maps(q_b, k_b, q_grid, k_grid, labels, neg_noise):
    q_b = np.ascontiguousarray(np.asarray(q_b, dtype=np.float32)).reshape(B, C, N)
    k_b = np.ascontiguousarray(np.asarray(k_b, dtype=np.float32)).reshape(B, C, N)
    q_grid = np.ascontiguousarray(np.asarray(q_grid, dtype=np.float32)).reshape(B, C, N)
    k_grid = np.ascontiguousarray(np.asarray(k_grid, dtype=np.float32)).reshape(B, C, N)
    labels = np.asarray(labels)
    neg_noise = np.asarray(neg_noise, dtype=np.float32)

    # negative-sample index prep (O(B^2), matches jnp argmax tie-breaking)
    mask = labels[None, :] != labels[:, None]
    scores = np.where(mask, neg_noise, -np.inf)
    neg_idx = np.argmax(scores, axis=1)
    kng = k_grid[neg_idx]  # [B, C, N]

    mt = N // 128
    cst_ind = np.zeros((C, mt, mt), dtype=np.float32)
    cst_indT = np.zeros((mt, mt, C), dtype=np.float32)
    for j in range(mt):
        cst_ind[:, j, j] = 1.0
        cst_indT[j, j, :] = 1.0
    cst_ind = cst_ind.reshape(C, mt * mt)
    cst_indT = np.ascontiguousarray(cst_indT.transpose(1, 0, 2)).reshape(mt, mt * C)

    in_maps = []
    for ci in range(NCORES):
        sl = slice(ci * SPC, (ci + 1) * SPC)
        in_maps.append({
            "qb": np.ascontiguousarray(q_b[sl]).reshape(SPC * C, N),
            "kb": np.ascontiguousarray(k_b[sl]).reshape(SPC * C, N),
            "qgt": np.ascontiguousarray(q_grid[sl].transpose(0, 2, 1)).reshape(SPC * N, C),
            "kgt": np.ascontiguousarray(k_grid[sl].transpose(0, 2, 1)).reshape(SPC * N, C),
            "kngt": np.ascontiguousarray(kng[sl].transpose(0, 2, 1)).reshape(SPC * N, C),
            "cst_ind": cst_ind,
            "cst_indT": cst_indT,
        })
    return in_maps


def kernel(q_b, k_b, q_grid, k_grid, labels, neg_noise):
    global LAST_EXEC_TIME_NS
    _ensure_ntff_hook()
    in_maps = make_in_maps(q_b, k_b, q_grid, k_grid, labels, neg_noise)
    nc = get_module()
    from concourse.bass_utils import run_bass_kernel_spmd
    res = run_bass_kernel_spmd(nc, in_maps, core_ids=list(range(NCORES)))
    LAST_EXEC_TIME_NS = res.exec_time_ns
    total = sum(float(res.results[i]["out"][0, 0]) for i in range(NCORES))
    return np.float32(total / float(B * N))



# revision 11
# speedup vs baseline: 1.8252x; 1.8252x over previous
"""Dense correspondence contrastive loss kernel for Trainium2 (8 NeuronCores).

Problem (B=32, C=64, N=1024 spatial positions per sample):
  - l2-normalize q_b/k_b/q_grid/k_grid along C
  - sim[b,i,j] = <qb_hat[b,:,i], kb_hat[b,:,j]>; idx = argmax_j sim
  - pos[b,i] = <qg_hat[b,:,i], kg_hat[b,:,idx[b,i]]> / 0.1
  - neg[b,i] = <qg_hat[b,:,i], kg_hat[neg_idx[b],:,i]> / 0.1
    (neg_idx from labels/neg_noise -- O(B^2) host-side index prep)
  - loss = mean(log(exp(pos)+exp(neg)+1e-6) - pos)

Sharding: data-parallel over batch, 4 samples per core.

v2 design (per core):
  - q_b normalization skipped (argmax invariant to per-row scale); k_b
    column norms rn[j] = 1/|k_b[:,j]| computed via PE colsum of squares
    into a partition-parallel [8,128] layout, then reshaped/broadcast to
    a [128,1024] SBUF tile with two small DMAs.
  - sim matmuls run on RAW fp32 operands in float32r mode (1 cyc/row at
    >=256 moving); no bf16 conversion passes.
  - single-pass fused argmax: a custom DVE op computes
    argmax_j(sim[i,j]*rn[j]) in ONE pass over PSUM (running scan-max +
    index select + MAX-accumulator), replacing the reduce_max + STT pair.
  - ONE batched indirect DMA per sample gathers all 8 m-chunks of
    matched k_grid rows (1024 descriptors, single SWDGE fixed cost).
  - grid tail: ACT squares + Pool products into a [128,2560] scratch,
    strided DVE reduces into kind-major accumulators, batched loss tail.
"""

import os
import numpy as np

B = 32
C = 64
N = 1024
NCORES = 8
SPC = B // NCORES          # samples per core
MT = N // 128              # 128-row m-tiles per sample
NT = SPC * MT              # accumulator columns per core
TEMP = 0.1
EPS_LOSS = 1e-6

USE_F32R = False           # fp32r sim matmuls rejected by BIR verifier
                           # ("not rounded to FP32r"); bf16 copies instead

LAST_EXEC_TIME_NS = None
_CACHE = {}


def _ensure_ntff_hook():
    """Some agent images ship only the antenv stub (no axon_hooks); bass_utils
    then crashes on `from antenv.axon_hooks import ...` when tracing under
    axon.  Install a functional shim wired to the libaxon ctypes hook so NTFF
    profiling (and exec_time_ns) works.  No-op when the real module exists."""
    import sys
    import types
    try:
        import antenv.axon_hooks  # noqa: F401
        return
    except ImportError:
        pass
    try:
        import antenv
    except ImportError:
        return
    mod = types.ModuleType("antenv.axon_hooks")
    mod._hook = None

    def set_axon_ntff_profile_hook(h):
        mod._hook = h

    def get_axon_ntff_profile_hook():
        return mod._hook

    mod.set_axon_ntff_profile_hook = set_axon_ntff_profile_hook
    mod.get_axon_ntff_profile_hook = get_axon_ntff_profile_hook
    sys.modules["antenv.axon_hooks"] = mod
    antenv.axon_hooks = mod
    try:
        from trn_agent_boot.trn_boot import _ntff_profile_via_ctypes
        hook = _ntff_profile_via_ctypes("/opt/axon/libaxon_pjrt.so")
        if hook is not None:
            mod._hook = hook
    except Exception:
        pass


def _register_argmax_op():
    """Register a custom DVE op: single-pass scaled argmax.

    out[k]    = select(v_k >= runmax(v)_k, k, -FLT_MAX),  v = in0*in1
    accum_out = max_k out[k]   (== argmax_k v; last index on exact ties,
                                but fp32 exact ties have ~0 probability)
    """
    from concourse import dve_ops
    from concourse.dve_spec import (
        Spec, lower, Src0, Src1, scan, Idx, select, AluOp, MaxNeg, _has_src1,
    )
    from concourse.dve_uop import DveOpSpec
    from concourse.dve_ops import DveOp

    name = "ARGMAX_SCALED_ANT"
    for op in dve_ops.OPS:
        if op.name == name:
            return op

    def ref(in0, in1, c0, c1, c2):
        p = in0.shape[0]
        a = np.asarray(in0, np.float32).reshape(p, -1)
        bmat = np.asarray(in1, np.float32).reshape(p, -1)
        v = a * bmat
        run = np.maximum.accumulate(v, axis=1)
        cond = v >= run
        idxs = np.arange(a.shape[1], dtype=np.float32)[None, :]
        out = np.where(cond, idxs, np.float32(-3.4028234663852886e38))
        acc = out.max(axis=1)
        return out.reshape(in0.shape), acc

    v = Src0 * Src1
    body = select(v >= scan(AluOp.MAX, v), Idx, MaxNeg)
    spec = Spec(body=body, accum=AluOp.MAX, reference=ref)

    row = max(dve_ops._SUB_OPCODE_FOR_NAME.values()) + 1
    assert row < 0x20
    dve_ops._SUB_OPCODE_FOR_NAME[name] = row
    shas = {}
    for ver in ("v3", "v4"):
        try:
            tmp = DveOpSpec(name=name, opcode=row, uops=lower(spec, ver=ver),
                            rd1_en=_has_src1(spec))
            shas[ver] = tmp.sha(ver)
        except Exception:
            pass
    op = DveOp(name, spec, subdim=False, uops_sha=shas)
    dve_ops.OPS.append(op)
    dve_ops.CUSTOM_DVE_SPECS[name] = spec
    return op


def _build_module():
    import concourse.bass as bass
    import concourse.bacc as bacc
    import concourse.tile as tile
    from concourse import mybir
    from contextlib import ExitStack

    argmax_op = _register_argmax_op()

    F32 = mybir.dt.float32
    F32R = mybir.dt.float32r
    BF16 = mybir.dt.bfloat16
    FP16 = mybir.dt.float16
    U32 = mybir.dt.uint32
    AX = mybir.AxisListType
    ALU = mybir.AluOpType
    ACTF = mybir.ActivationFunctionType

    nc = bacc.Bacc("TRN2", target_bir_lowering=False, debug=False,
                   num_devices=NCORES)

    qb_d = nc.dram_tensor("qb", [SPC * C, N], F32, kind="ExternalInput")
    kb_d = nc.dram_tensor("kb", [SPC * C, N], F32, kind="ExternalInput")
    qgt_d = nc.dram_tensor("qgt", [SPC * N, C], F32, kind="ExternalInput")
    kgt_d = nc.dram_tensor("kgt", [SPC * N, C], F32, kind="ExternalInput")
    kngt_d = nc.dram_tensor("kngt", [SPC * N, C], F32, kind="ExternalInput")
    ind_d = nc.dram_tensor("cst_ind", [C, MT * MT], BF16, kind="ExternalInput")
    out_d = nc.dram_tensor("out", [1, 1], F32, kind="ExternalOutput")

    with tile.TileContext(nc) as tc, ExitStack() as ctx:
        const = ctx.enter_context(tc.tile_pool(name="const", bufs=1))
        accum = ctx.enter_context(tc.tile_pool(name="accum", bufs=1))
        io = ctx.enter_context(tc.tile_pool(name="io", bufs=2))
        qg_p = ctx.enter_context(tc.tile_pool(name="qg", bufs=3))
        mt_p = ctx.enter_context(tc.tile_pool(name="mt", bufs=4))
        scr = ctx.enter_context(tc.tile_pool(name="scr", bufs=2))
        ps_sim = ctx.enter_context(tc.tile_pool(name="ps_sim", bufs=2, space="PSUM"))
        ps_rnb = ctx.enter_context(tc.tile_pool(name="ps_rnb", bufs=1, space="PSUM"))
        ps_aux = ctx.enter_context(tc.tile_pool(name="ps_aux", bufs=2, space="PSUM"))

        ones128 = const.tile([128, 1], F32)
        nc.vector.memset(ones128[:], 1.0)
        ones1x128 = const.tile([1, 128], F32 if USE_F32R else BF16)
        nc.vector.memset(ones1x128[:], 1.0)
        b24 = const.tile([MT, 1], F32)
        nc.vector.memset(b24[:], 1e-24)
        b24t = const.tile([128, 1], F32)
        nc.vector.memset(b24t[:], 1e-24)
        # chunk-indicator weights: colsum of sq chunk j lands in PSUM
        # partition j (PE output base partition must be 0)
        ind_sb = const.tile([C, MT * MT], BF16)
        nc.sync.dma_start(ind_sb[:], ind_d[:, :])
        inds = [ind_sb[:, j * MT:(j + 1) * MT] for j in range(MT)]

        # kind-major accumulators for the batched loss tail
        ssq3 = accum.tile([128, 3 * NT], F32, tag="ssq3")   # qg | kga | kng
        dd = accum.tile([128, 2 * NT], F32, tag="dd")       # dps | dns

        import concourse.bass as bass_mod

        def emit_load(b):
            st = {}
            kb_t = io.tile([C, N], F32, tag="kb")
            nc.sync.dma_start(kb_t[:], kb_d[b * C:(b + 1) * C, :])
            qb_t = io.tile([C, N], F32, tag="qb")
            nc.sync.dma_start(qb_t[:], qb_d[b * C:(b + 1) * C, :])
            qgs = qg_p.tile([128, MT * C], F32, tag="qg")
            nc.sync.dma_start(
                qgs[:], qgt_d[b * N:(b + 1) * N, :].rearrange("(m p) c -> p m c", p=128))
            kngs = qg_p.tile([128, MT * C], F32, tag="kng")
            nc.sync.dma_start(
                kngs[:], kngt_d[b * N:(b + 1) * N, :].rearrange("(m p) c -> p m c", p=128))
            st["kb_t"], st["qb_t"], st["qgs"], st["kngs"] = kb_t, qb_t, qgs, kngs
            return st

        def emit_norm(b, st):
            # k_b column norms -> reciprocal -> [128, N] broadcast tile for
            # the fused argmax multiplier
            sq = io.tile([C, N], BF16, tag="sq")
            nc.scalar.activation(sq[:], st["kb_t"][:], ACTF.Square)
            ssq_ps = ps_aux.tile([MT, 128], F32, tag="aux")
            for j in range(MT):
                nc.tensor.matmul(ssq_ps[:], inds[j],
                                 sq[:, j * 128:(j + 1) * 128],
                                 start=(j == 0), stop=(j == MT - 1))
            rn_s = io.tile([MT, 128], F32, tag="rn_s")
            nc.scalar.activation(rn_s[:], ssq_ps[:], ACTF.Sqrt, bias=b24[:])
            rn8 = io.tile([MT, 128], F32, tag="rn8")
            nc.vector.reciprocal(rn8[:], rn_s[:])
            # [8,128] partition-parallel -> [1, 1024] row (DMA reshape), then
            # K=1 matmul broadcasts the row over all 128 PSUM partitions
            rnb_ps = ps_rnb.tile([128, N], F32, tag="rnb")
            if USE_F32R:
                rnrow = io.tile([1, N], F32, tag="rnrow")
                nc.sync.dma_start(rnrow[0:1, :], rn8[:, :])
                for h in range(2):
                    nc.tensor.matmul(rnb_ps[:, h * 512:(h + 1) * 512],
                                     ones1x128[:].bitcast(F32R),
                                     rnrow[0:1, h * 512:(h + 1) * 512].bitcast(F32R),
                                     start=True, stop=True)
            else:
                rn8h = io.tile([MT, 128], BF16, tag="rn8h")
                nc.scalar.activation(rn8h[:], rn8[:], ACTF.Copy)
                rnrow = io.tile([1, N], BF16, tag="rnrow")
                nc.sync.dma_start(rnrow[0:1, :], rn8h[:, :])
                for h in range(2):
                    nc.tensor.matmul(rnb_ps[:, h * 512:(h + 1) * 512],
                                     ones1x128[:], rnrow[0:1, h * 512:(h + 1) * 512],
                                     start=True, stop=True)
                qb_bf = io.tile([C, N], BF16, tag="qb_bf")
                nc.scalar.activation(qb_bf[:], st["qb_t"][:], ACTF.Copy)
                kb_bf = io.tile([C, N], BF16, tag="kb_bf")
                nc.scalar.activation(kb_bf[:], st["kb_t"][:], ACTF.Copy)
                st["qb_bf"], st["kb_bf"] = qb_bf, kb_bf
            # custom-dve ops may read only ONE non-scalar input from PSUM;
            # sim stays in PSUM, so the norm row goes via SBUF
            rnb_sb = io.tile([128, N], F32, tag="rnb_sb")
            nc.scalar.activation(rnb_sb[:], rnb_ps[:], ACTF.Copy)
            st["rnb"] = rnb_sb
            st["idxf"] = mt_p.tile([128, MT], F32, tag="idxf", name=f"idxf{b}")
            return st

        def emit_mtile(b, m, st):
            sim_ps = ps_sim.tile([128, N], F32, tag="sim")
            if USE_F32R:
                lhs = st["qb_t"][:, m * 128:(m + 1) * 128].bitcast(F32R)
                rhs0 = st["kb_t"][:, 0:512].bitcast(F32R)
                rhs1 = st["kb_t"][:, 512:N].bitcast(F32R)
            else:
                lhs = st["qb_bf"][:, m * 128:(m + 1) * 128]
                rhs0 = st["kb_bf"][:, 0:512]
                rhs1 = st["kb_bf"][:, 512:N]
            nc.tensor.matmul(sim_ps[:, 0:512], lhs, rhs0, start=True, stop=True)
            nc.tensor.matmul(sim_ps[:, 512:N], lhs, rhs1, start=True, stop=True)
            scrap = scr.tile([128, N], FP16, tag="scrap")
            nc.vector._custom_dve(
                argmax_op, out=scrap[:], in0=sim_ps[:], in1=st["rnb"][:],
                accum_out=st["idxf"][:, m:m + 1])

        def emit_gather(b, st, mlo, mhi):
            idxc = mt_p.tile([128, mhi - mlo], F32, tag="idxc")
            nc.vector.tensor_scalar(idxc[:], st["idxf"][:, mlo:mhi],
                                    float(b * N), 0.0, op0=ALU.add, op1=ALU.add)
            idxu = mt_p.tile([128, mhi - mlo], U32, tag="idxu")
            nc.vector.tensor_copy(idxu[:], idxc[:])
            nc.gpsimd.indirect_dma_start(
                st["kgas"][:, mlo * C:mhi * C], None, kgt_d.ap(),
                bass_mod.IndirectOffsetOnAxis(ap=idxu[:, 0:mhi - mlo], axis=0))

        def emit_early_prep(b, st):
            # gather-independent pieces: qg^2, kng^2, qg*kng and their reduces
            big5 = scr.tile([128, 5 * MT * C], F32, tag="big5", name=f"big5{b}")
            st["big5"] = big5
            nc.scalar.activation(big5[:, 0:512], st["qgs"][:], ACTF.Square)
            nc.scalar.activation(big5[:, 512:1024], st["kngs"][:], ACTF.Square)
            nc.gpsimd.tensor_mul(big5[:, 2048:2560], st["qgs"][:], st["kngs"][:])
            nc.vector.tensor_reduce(
                ssq3[:, b * MT:(b + 1) * MT],
                big5[:, 0:512].rearrange("p (m c) -> p m c", c=C),
                axis=AX.X, op=ALU.add)
            nc.vector.tensor_reduce(
                ssq3[:, 2 * NT + b * MT:2 * NT + (b + 1) * MT],
                big5[:, 512:1024].rearrange("p (m c) -> p m c", c=C),
                axis=AX.X, op=ALU.add)
            nc.vector.tensor_reduce(
                dd[:, NT + b * MT:NT + (b + 1) * MT],
                big5[:, 2048:2560].rearrange("p (m c) -> p m c", c=C),
                axis=AX.X, op=ALU.add)

        def emit_late_prep(b, st, mlo, mhi):
            # gather-dependent pieces: kga^2, qg*kga and their reduces
            big5 = st["big5"]
            lo, hi = mlo * C, mhi * C
            nc.scalar.activation(big5[:, 1024 + lo:1024 + hi],
                                 st["kgas"][:, lo:hi], ACTF.Square)
            nc.gpsimd.tensor_mul(big5[:, 1536 + lo:1536 + hi],
                                 st["qgs"][:, lo:hi], st["kgas"][:, lo:hi])
            nc.vector.tensor_reduce(
                ssq3[:, NT + b * MT + mlo:NT + b * MT + mhi],
                big5[:, 1024 + lo:1024 + hi].rearrange("p (m c) -> p m c", c=C),
                axis=AX.X, op=ALU.add)
            nc.vector.tensor_reduce(
                dd[:, b * MT + mlo:b * MT + mhi],
                big5[:, 1536 + lo:1536 + hi].rearrange("p (m c) -> p m c", c=C),
                axis=AX.X, op=ALU.add)

        # software-pipelined emission
        st = emit_load(0)
        emit_norm(0, st)
        states = {0: st}
        for b in range(SPC):
            cur = states.pop(b)
            cur["kgas"] = qg_p.tile([128, MT * C], F32, tag="kga", name=f"kgas{b}")
            last = b == SPC - 1
            for m in range(MT):
                emit_mtile(b, m, cur)
                if m == 0 and not last:
                    states[b + 1] = emit_load(b + 1)
                if m == 1 and not last:
                    emit_norm(b + 1, states[b + 1])
                if m == 2:
                    emit_early_prep(b, cur)
                if m == 5 and b > 0:
                    prev = states.pop(("done", b - 1))
                    emit_late_prep(b - 1, prev, 0, MT)
                if last and m == 3:
                    # drain shortening: first half of the last sample's
                    # gather + tail prep overlaps its remaining m-tiles
                    emit_gather(b, cur, 0, 4)
            if last:
                emit_gather(b, cur, 4, MT)
                emit_late_prep(b, cur, 0, 4)
                emit_late_prep(b, cur, 4, MT)
            else:
                emit_gather(b, cur, 0, MT)
                states[("done", b)] = cur

        # batched loss tail; the 1/TEMP=10 factor rides as the stt immediate
        ra_s = accum.tile([128, 3 * NT], F32, tag="ra_s")
        nc.scalar.activation(ra_s[:], ssq3[:], ACTF.Sqrt, bias=b24t[:])
        ra = accum.tile([128, 3 * NT], F32, tag="ra")
        nc.vector.reciprocal(ra[:], ra_s[:])

        t1 = accum.tile([128, NT], F32, tag="t1")
        nc.vector.tensor_mul(t1[:], dd[:, 0:NT], ra[:, 0:NT])
        pos = accum.tile([128, NT], F32, tag="pos")
        nc.vector.scalar_tensor_tensor(pos[:], t1[:], 10.0, ra[:, NT:2 * NT],
                                       op0=ALU.mult, op1=ALU.mult)
        t2 = accum.tile([128, NT], F32, tag="t2")
        nc.vector.tensor_mul(t2[:], dd[:, NT:2 * NT], ra[:, 0:NT])
        ngv = accum.tile([128, NT], F32, tag="ngv")
        nc.vector.scalar_tensor_tensor(ngv[:], t2[:], 10.0, ra[:, 2 * NT:3 * NT],
                                       op0=ALU.mult, op1=ALU.mult)

        ep = accum.tile([128, NT], F32, tag="ep")
        nc.scalar.activation(ep[:], pos[:], ACTF.Exp)
        en = accum.tile([128, NT], F32, tag="en")
        nc.scalar.activation(en[:], ngv[:], ACTF.Exp)
        ssum = accum.tile([128, NT], F32, tag="ssum")
        nc.vector.scalar_tensor_tensor(ssum[:], ep[:], EPS_LOSS, en[:],
                                       op0=ALU.add, op1=ALU.add)
        lg = accum.tile([128, NT], F32, tag="lg")
        nc.scalar.activation(lg[:], ssum[:], ACTF.Ln)
        li = accum.tile([128, NT], F32, tag="li")
        nc.vector.tensor_sub(li[:], lg[:], pos[:])
        lsum = accum.tile([128, 1], F32, tag="lsum")
        nc.vector.reduce_sum(lsum[:], li[:], axis=AX.X)

        tot_ps = ps_aux.tile([1, 1], F32, tag="aux")
        nc.tensor.matmul(tot_ps[:], lsum[:], ones128[:], start=True, stop=True)
        outt = mt_p.tile([1, 1], F32, tag="outt")
        nc.scalar.activation(outt[:], tot_ps[:], ACTF.Copy)
        nc.sync.dma_start(out_d[:, :], outt[:])

    nc.compile()
    return nc


def get_module():
    if "nc" not in _CACHE:
        _CACHE["nc"] = _build_module()
    return _CACHE["nc"]


def make_in_maps(q_b, k_b, q_grid, k_grid, labels, neg_noise):
    import ml_dtypes

    q_b = np.ascontiguousarray(np.asarray(q_b, dtype=np.float32)).reshape(B, C, N)
    k_b = np.ascontiguousarray(np.asarray(k_b, dtype=np.float32)).reshape(B, C, N)
    q_grid = np.ascontiguousarray(np.asarray(q_grid, dtype=np.float32)).reshape(B, C, N)
    k_grid = np.ascontiguousarray(np.asarray(k_grid, dtype=np.float32)).reshape(B, C, N)
    labels = np.asarray(labels)
    neg_noise = np.asarray(neg_noise, dtype=np.float32)

    # negative-sample index prep (O(B^2), matches jnp argmax tie-breaking)
    mask = labels[None, :] != labels[:, None]
    scores = np.where(mask, neg_noise, -np.inf)
    neg_idx = np.argmax(scores, axis=1)
    kng = k_grid[neg_idx]  # [B, C, N]

    mt = N // 128
    cst_ind = np.zeros((C, mt, mt), dtype=np.float32)
    for j in range(mt):
        cst_ind[:, j, j] = 1.0
    cst_ind = cst_ind.reshape(C, mt * mt).astype(ml_dtypes.bfloat16)

    in_maps = []
    for ci in range(NCORES):
        sl = slice(ci * SPC, (ci + 1) * SPC)
        in_maps.append({
            "qb": np.ascontiguousarray(q_b[sl]).reshape(SPC * C, N),
            "kb": np.ascontiguousarray(k_b[sl]).reshape(SPC * C, N),
            "qgt": np.ascontiguousarray(q_grid[sl].transpose(0, 2, 1)).reshape(SPC * N, C),
            "kgt": np.ascontiguousarray(k_grid[sl].transpose(0, 2, 1)).reshape(SPC * N, C),
            "kngt": np.ascontiguousarray(kng[sl].transpose(0, 2, 1)).reshape(SPC * N, C),
            "cst_ind": cst_ind,
        })
    return in_maps


def kernel(q_b, k_b, q_grid, k_grid, labels, neg_noise):
    global LAST_EXEC_TIME_NS
    _ensure_ntff_hook()
    in_maps = make_in_maps(q_b, k_b, q_grid, k_grid, labels, neg_noise)
    nc = get_module()
    from concourse.bass_utils import run_bass_kernel_spmd
    res = run_bass_kernel_spmd(nc, in_maps, core_ids=list(range(NCORES)))
    LAST_EXEC_TIME_NS = res.exec_time_ns
    total = sum(float(res.results[i]["out"][0, 0]) for i in range(NCORES))
    return np.float32(total / float(B * N))
